# revision 11
# baseline (speedup 1.0000x reference)
"""MoE adapter (router + rank-16 expert adapters) Trainium2 Bass kernel, v2.

Math: with w[t,e] the dense (zero for non-top2) UNNORMALIZED top-2 gates
(kp = exp values of the top-2, zero elsewhere) and den[t] = sum_e kp[t,e]:
  out[t,:] = (1/den[t]) * [ (kp_expand ⊙ relu(x@WdFlat + bdFlat)) @ WuFlat + kp @ bu ]
The 1/den is folded into the PSUM->SBUF evacuation of the final output
(per-partition scalar multiply), so gates are never normalized explicitly.
exp() needs no max-subtraction: |logit| <= ||x_row||*||Wr_col|| + |br| < 40,
so exp stays finite in fp32 and top-2 ratios are shift-invariant anyway.

Sharding: pure data parallel, tokens split 8 ways, weights replicated.

Per-core, per 512-token stripe, software-pipelined across stripes
(front(s) = transpose/router/down; tail(s-1) = gate-consumers/up/bias/out):
  PE:  32 x-transposes (f32r), 8 router MMs, 8 down MMs, 4 logit transposes,
       4 kp transposes, 1 sel MM (gate broadcast), 16 up/bias MMs.
  ACT: 4 x^T evacs, relu+bias, exp, wt evac, plg evac, 4 out evacs.
  DVE: 4 x^T evacs, gate math (~8 small ops), hp gate-mul, 4 out evacs.
  DMA: 4x 512KB in, 4x 512KB out  (the ~11.2us/stripe roofline).
"""

import sys

sys.path.insert(0, "/opt/trn_rl_repo")

from contextlib import ExitStack

import numpy as np

import concourse.bacc as bacc
import concourse.bass as bass
import concourse.mybir as mybir
import concourse.tile as tile

F32 = mybir.dt.float32
F32R = mybir.dt.float32r
BF16 = mybir.dt.bfloat16
ALU = mybir.AluOpType
ACTF = mybir.ActivationFunctionType

B, S, D = 8, 4096, 1024
E, R, TOP_K = 8, 16, 2
ER = E * R  # 128
N_CORES = 8
T_CORE = B * S // N_CORES  # 4096 tokens per core
STRIPE = 512
NBLK = STRIPE // 128  # 4
KC = D // 128  # 8 k-chunks


def _build_program_v2(t_core: int = T_CORE, fast_math: bool = True, time_loops: int = 1, cfg: dict | None = None):
    nc = bacc.Bacc("TRN2", target_bir_lowering=False, debug=False)
    CF = {
        "xin_bufs": 8,
        "xtp_bufs": 2,
        "pbig_bufs": 3,
        "outp_bufs": 4,
        "smal_bufs": 2,
        "stt": True,          # use fused scalar_tensor_tensor for top-2 masks
        "act_scale_ap": True,  # ACT Copy with per-partition scale AP for out evac
        "skew": 2,             # chunks of evac lag before router/down start
        "evac_pat": "avavavav",  # xt-evac engine per chunk: a=ACT v=DVE
        "po_pat": "vavavava",    # out-evac engine per (blk,half)
        "bcast_mask": False,     # use broadcast-AP tensor_tensor for top-2 masks
        "dma_pair": False,       # pair 128-row blocks into 1MB DMAs
        "out_dma_eng": "sync",   # ring for out-DMAs: scalar=ACT-HWDGE, sync=SP
        "ct_router": False,      # col-tiled router: concurrent col-group MMs
        "ct_groups": 4,          # how many 32-col groups to use (3 or 4)
        "ta_c": 0,               # chunk index at which prev-stripe tail_a emits
    }
    CF.update(cfg or {})

    phase = CF.get("phase", "")
    x = nc.dram_tensor("x", [t_core, D], F32R, kind="ExternalInput").ap()
    wds = nc.dram_tensor("wds", [128, D], F32R, kind="ExternalInput").ap()
    wrs = nc.dram_tensor("wrs", [128, KC * E], F32R, kind="ExternalInput").ap()
    wus = nc.dram_tensor("wus", [ER, D], BF16, kind="ExternalInput").ap()
    bus = nc.dram_tensor("bus", [E, D], BF16, kind="ExternalInput").ap()
    bds = nc.dram_tensor("bds", [128, 1], F32, kind="ExternalInput").ap()
    brb = nc.dram_tensor("brb", [128, NBLK * E], F32, kind="ExternalInput").ap()
    i128b = nc.dram_tensor("i128b", [128, 128], BF16, kind="ExternalInput").ap()
    i128r = nc.dram_tensor("i128r", [128, 128], F32R, kind="ExternalInput").ap()
    sel = nc.dram_tensor("sel", [E, ER], BF16, kind="ExternalInput").ap()
    csum = nc.dram_tensor("csum", [128, E], F32R, kind="ExternalInput").ap()
    out = nc.dram_tensor(
        "out", [t_core, D], F32R if phase in ("T", "D") else F32, kind="ExternalOutput"
    ).ap()

    n_stripes = t_core // STRIPE
    assert t_core % STRIPE == 0

    with tile.TileContext(nc) as tc, ExitStack() as ctx:
        const = ctx.enter_context(tc.tile_pool(name="const", bufs=1))
        xin = ctx.enter_context(tc.tile_pool(name="xin", bufs=CF["xin_bufs"]))
        xtp = ctx.enter_context(tc.tile_pool(name="xt", bufs=CF["xtp_bufs"]))
        hsp = ctx.enter_context(tc.tile_pool(name="hs", bufs=2))
        hpp = ctx.enter_context(tc.tile_pool(name="hp", bufs=2))
        smal = ctx.enter_context(tc.tile_pool(name="smal", bufs=CF["smal_bufs"]))
        outp = ctx.enter_context(tc.tile_pool(name="outsb", bufs=CF["outp_bufs"]))
        # PSUM (8 banks): pbig 3 (x-transpose tiles & up-output tiles,
        # disjoint phases) + plg 1 + ph 2 + psmA 2 (pwt/pwb/plgtm).
        pbig = ctx.enter_context(tc.tile_pool(name="pbig", bufs=CF["pbig_bufs"], space="PSUM"))
        plgp = ctx.enter_context(tc.tile_pool(name="plg", bufs=1, space="PSUM"))
        php = ctx.enter_context(tc.tile_pool(name="ph", bufs=2, space="PSUM"))
        psmA = ctx.enter_context(tc.tile_pool(name="psmA", bufs=2, space="PSUM"))

        # ---- one-time constant loads ----
        i128b_t = const.tile([128, 128], BF16)
        nc.sync.dma_start(i128b_t[:], i128b)
        i128r_t = const.tile([128, 128], F32R)
        nc.sync.dma_start(i128r_t[:], i128r)
        def load_x(tok0):
            if CF["dma_pair"]:
                xts = []
                for p in range(NBLK // 2):
                    xb = xin.tile([128, 2 * D], F32R, tag="xin")
                    src = x[tok0 + p * 256 : tok0 + (p + 1) * 256, :].rearrange(
                        "(c t) d -> t c d", c=2
                    )
                    nc.sync.dma_start(xb[:].rearrange("t (c d) -> t c d", d=D), src)
                    xts.append(xb[:, 0:D])
                    xts.append(xb[:, D : 2 * D])
                return xts
            xts = []
            for b in range(NBLK):
                xb = xin.tile([128, D], F32R, tag="xin")
                nc.sync.dma_start(xb[:], x[tok0 + b * 128 : tok0 + (b + 1) * 128, :])
                xts.append(xb[:])
            return xts

        pre_x = []
        if time_loops == 1:
            pre_x.extend(load_x(0))
        wds_t = const.tile([128, D], F32R)
        nc.sync.dma_start(wds_t[:], wds)
        wrs_t = const.tile([128, KC * E], F32R)
        nc.sync.dma_start(wrs_t[:], wrs)
        wus_t = const.tile([ER, D], BF16)
        nc.sync.dma_start(wus_t[:], wus)
        bus_t = const.tile([E, D], BF16)
        nc.sync.dma_start(bus_t[:], bus)
        bds_t = const.tile([128, 1], F32)
        nc.sync.dma_start(bds_t[:], bds)
        brb_t = const.tile([128, NBLK * E], F32)
        nc.sync.dma_start(brb_t[:], brb)
        sel_t = const.tile([E, ER], BF16)
        nc.sync.dma_start(sel_t[:], sel)
        if CF["ct_router"]:
            csum_t = const.tile([128, E], F32R)
            nc.sync.dma_start(csum_t[:], csum)
            plg128 = plgp.tile([128, STRIPE], F32, tag="plg")
            nc.vector.memset(plg128[:], 0.0)

        def v3(ap):
            return ap.rearrange("p (b e) -> p b e", e=E)

        # ---------- tail of stripe sp (gate-consumers, up+bias, out) ----------
        # split into tail_a (emitted early in the next stripe's front) and
        # tail_b (up/bias matmuls + out, emitted after the next front's MMs).
        def tail_a(st):
            if st is None:
                return
            kp, dinv = st["kp"], st["dinv"]
            # kp^T blocks -> pwt [8, 512]
            pwt = psmA.tile([E, STRIPE], BF16, tag="sm")
            for b in range(NBLK):
                nc.tensor.transpose(
                    pwt[:, b * 128 : (b + 1) * 128],
                    kp[:, b * E : (b + 1) * E],
                    i128b_t[:],
                )
            wt = smal.tile([E, STRIPE], BF16, tag="wt")
            nc.scalar.copy(wt[:], pwt[:])
            # broadcast gates to er rows: pb[16e+r, t] = kp[t, e]
            pb = psmA.tile([128, STRIPE], F32, tag="sm")
            nc.tensor.matmul(pb[:], sel_t[:], wt[:], start=True, stop=True)
            st["wt"] = wt
            st["pb"] = pb

        def tail_relu(st):
            if st is None:
                return
            hs = hsp.tile([128, STRIPE], F32R)
            nc.scalar.activation(hs[:], st["ph"][:], ACTF.Relu, bias=bds_t[:, 0:1])
            st["hs"] = hs

        def tail_hp(st):
            if st is None:
                return
            hp = hpp.tile([128, STRIPE], BF16)
            nc.vector.tensor_mul(hp[:], st["hs"][:], st["pb"][:])
            st["hp"] = hp

        def tail_b(st):
            if st is None:
                return
            tok0 = st["s"] * STRIPE
            hp, wt, dinv = st["hp"], st["wt"], st["dinv"]
            pair = CF["dma_pair"]
            odma = nc.scalar.dma_start if CF["out_dma_eng"] == "scalar" else nc.sync.dma_start
            nb_per = 2 if pair else 1
            for bp in range(NBLK // nb_per):
                osb = outp.tile([128, nb_per * D], F32, tag="osb")
                for bi in range(nb_per):
                    b = bp * nb_per + bi
                    for h2 in range(2):
                        po = pbig.tile([128, 512], F32, tag="pt")
                        nc.tensor.matmul(
                            po[:],
                            hp[:, b * 128 : (b + 1) * 128],
                            wus_t[:, h2 * 512 : (h2 + 1) * 512],
                            start=True,
                            stop=False,
                            skip_group_check=True,
                        )
                        nc.tensor.matmul(
                            po[:],
                            wt[:, b * 128 : (b + 1) * 128],
                            bus_t[:, h2 * 512 : (h2 + 1) * 512],
                            start=False,
                            stop=True,
                            skip_group_check=True,
                        )
                        dst = osb[:, bi * D + h2 * 512 : bi * D + (h2 + 1) * 512]
                        dv = dinv[:, b : b + 1]
                        if CF["po_pat"][b * 2 + h2] == "v" or not CF["act_scale_ap"]:
                            nc.vector.tensor_scalar_mul(dst, po[:], dv)
                        else:
                            nc.scalar.activation(dst, po[:], ACTF.Copy, scale=dv)
                if pair:
                    dstap = out[tok0 + bp * 256 : tok0 + (bp + 1) * 256, :].rearrange(
                        "(c t) d -> t c d", c=2
                    )
                    odma(dstap, osb[:].rearrange("t (c d) -> t c d", d=D))
                else:
                    dstap = out[tok0 + bp * 128 : tok0 + (bp + 1) * 128, :]
                    odma(dstap, osb[:])

        # ---------- front of stripe s + interleaved tail of prev ----------
        def emit_stripe(s, prev):
            tok0 = s * STRIPE
            # x DMAs
            if s == 0 and pre_x:
                xts = pre_x[:]
                pre_x.clear()
            else:
                xts = load_x(tok0)

            if phase == "D":
                # DMA-only roofline microbenchmark: bounce x straight back out
                for b in range(NBLK):
                    nc.sync.dma_start(
                        out[tok0 + b * 128 : tok0 + (b + 1) * 128, :], xts[b]
                    )
                return None

            xt_all = xtp.tile([128, KC * STRIPE], F32R)

            def xtc(c):
                return xt_all[:, c * STRIPE : (c + 1) * STRIPE]

            ct = CF["ct_router"]
            G = CF["ct_groups"]
            if not ct:
                plg = plgp.tile([E, STRIPE], F32, tag="plg")
            ph = php.tile([128, STRIPE], F32, tag="ph")
            skew = CF["skew"]

            def router_round(r):
                # G concurrent col-group matmuls (chunks r*G .. r*G+G-1)
                for g in range(G):
                    c = r * G + g
                    if c >= KC:
                        return
                    nc.tensor.matmul(
                        plg128[32 * g : 32 * g + E, :],
                        wrs_t[:, c * E : (c + 1) * E],
                        xtc(c),
                        start=(c == 0),
                        stop=(c == KC - 1),
                        skip_group_check=True,
                        tile_position=(0, 32 * g),
                    )

            rounds_done = set()

            def mm_pair(c):
                if ct:
                    # router handled in rounds; emit a round once its chunks
                    # are all evacuated
                    if (c + 1) % G == 0:
                        router_round(c // G)
                        rounds_done.add(c // G)
                else:
                    nc.tensor.matmul(
                        plg[:],
                        wrs_t[:, c * E : (c + 1) * E],
                        xtc(c),
                        start=(c == 0),
                        stop=(c == KC - 1),
                        skip_group_check=True,
                    )
                nc.tensor.matmul(
                    ph[:],
                    wds_t[:, c * 128 : (c + 1) * 128],
                    xtc(c),
                    start=(c == 0),
                    stop=(c == KC - 1),
                    skip_group_check=True,
                )

            for c in range(KC):
                pt = pbig.tile([128, STRIPE], F32R, tag="pt")
                for b in range(NBLK):
                    nc.tensor.transpose(
                        pt[:, b * 128 : (b + 1) * 128],
                        xts[b][:, c * 128 : (c + 1) * 128],
                        i128r_t[:],
                    )
                # alternate evac engine
                if CF["evac_pat"][c] == "a":
                    nc.scalar.copy(xtc(c), pt[:])
                else:
                    nc.vector.tensor_copy(xtc(c), pt[:])
                if phase == "T":
                    continue
                if c == CF["ta_c"]:
                    tail_a(prev)       # PE: 4 kp-transposes + sel MM
                if c == CF["ta_c"] + 1:
                    tail_relu(prev)    # ACT
                if c == CF["ta_c"] + 2:
                    tail_hp(prev)      # DVE
                if c >= skew:
                    mm_pair(c - skew)
            if phase == "T":
                # microbenchmark: DMA out the transposed data directly
                # (same 4x512KB out traffic as the real pipeline; values garbage)
                for j in range(NBLK):
                    nc.sync.dma_start(
                        out[tok0 + j * 128 : tok0 + (j + 1) * 128, :],
                        xt_all[:, j * 2 * STRIPE : (j + 1) * 2 * STRIPE],
                    )
                return None
            for c in range(KC - skew, KC):
                mm_pair(c)

            if ct:
                for r in range((KC + G - 1) // G):
                    if r not in rounds_done:
                        router_round(r)

            st = {"s": s, "ph": ph}

            # evacuate router logits NOW so the ACT op sits ahead of the
            # prev-stripe out-evacs in the queue; the PE-side logit
            # transposes stay after tail_b (no PE stall, gate chain ~1.2us
            # earlier)
            if CF.get("hoist_lgt", True):
                if ct:
                    lgt = smal.tile([128, STRIPE], F32R, tag="lgt")
                    nc.scalar.copy(lgt[:], plg128[:])
                else:
                    lgt = smal.tile([E, STRIPE], F32R, tag="lgt")
                    nc.scalar.copy(lgt[:], plg[:])
                lgt_done = True
            else:
                lgt_done = False

            # up/bias + out of previous stripe (PE keeps busy while this
            # stripe's logits are evacuated and gated)
            tail_b(prev)

            # ---- router logits -> token-major ----
            if ct:
                if not lgt_done:
                    lgt = smal.tile([128, STRIPE], F32R, tag="lgt")
                    nc.scalar.copy(lgt[:], plg128[:])
                plgtm = psmA.tile([128, NBLK * E], F32, tag="sm")
                for b in range(NBLK):
                    nc.tensor.matmul(
                        plgtm[:, b * E : (b + 1) * E],
                        lgt[:, b * 128 : (b + 1) * 128],
                        csum_t[:],
                        start=True,
                        stop=True,
                        skip_group_check=True,
                    )
            else:
                if not lgt_done:
                    lgt = smal.tile([E, STRIPE], F32R, tag="lgt")
                    nc.scalar.copy(lgt[:], plg[:])
                plgtm = psmA.tile([128, NBLK * E], F32R, tag="sm")
                for b in range(NBLK):
                    nc.tensor.transpose(
                        plgtm[:, b * E : (b + 1) * E],
                        lgt[0:E, b * 128 : (b + 1) * 128],
                        i128r_t[:E, :E],
                    )

            # ---- top-2 gate math (unnormalized; 1/den deferred to out evac) ----
            lg2 = smal.tile([128, NBLK * E], F32, tag="lg2")
            nc.vector.tensor_add(lg2[:], plgtm[:], brb_t[:])
            ex = smal.tile([128, NBLK * E], F32, tag="ex")
            nc.scalar.activation(ex[:], lg2[:], ACTF.Exp)
            m1 = smal.tile([128, NBLK], F32, tag="m1")
            nc.vector.reduce_max(out=m1[:], in_=v3(ex[:]), axis=mybir.AxisListType.X)
            exm = smal.tile([128, NBLK * E], F32, tag="exm")
            kp = smal.tile([128, NBLK * E], BF16, tag="kp")
            m2 = smal.tile([128, NBLK], F32, tag="m2")
            if CF["bcast_mask"]:
                m1b = m1[:].unsqueeze(2).broadcast_to([128, NBLK, E])
                nc.vector.tensor_tensor(
                    out=v3(exm[:]), in0=v3(ex[:]), in1=m1b, op=ALU.is_lt
                )
                nc.vector.tensor_mul(exm[:], exm[:], ex[:])
                nc.vector.reduce_max(out=m2[:], in_=v3(exm[:]), axis=mybir.AxisListType.X)
                m2b = m2[:].unsqueeze(2).broadcast_to([128, NBLK, E])
                nc.vector.tensor_tensor(
                    out=v3(kp[:]), in0=v3(ex[:]), in1=m2b, op=ALU.is_ge
                )
                nc.vector.tensor_mul(kp[:], kp[:], ex[:])
            elif CF["stt"]:
                for b in range(NBLK):
                    eb = ex[:, b * E : (b + 1) * E]
                    nc.vector.scalar_tensor_tensor(
                        exm[:, b * E : (b + 1) * E], eb, m1[:, b : b + 1], eb,
                        op0=ALU.is_lt, op1=ALU.mult,
                    )
                nc.vector.reduce_max(out=m2[:], in_=v3(exm[:]), axis=mybir.AxisListType.X)
                for b in range(NBLK):
                    eb = ex[:, b * E : (b + 1) * E]
                    nc.vector.scalar_tensor_tensor(
                        kp[:, b * E : (b + 1) * E], eb, m2[:, b : b + 1], eb,
                        op0=ALU.is_ge, op1=ALU.mult,
                    )
            else:
                for b in range(NBLK):
                    nc.vector.tensor_scalar(
                        out=exm[:, b * E : (b + 1) * E],
                        in0=ex[:, b * E : (b + 1) * E],
                        scalar1=m1[:, b : b + 1], scalar2=None, op0=ALU.is_lt,
                    )
                nc.vector.tensor_mul(exm[:], exm[:], ex[:])
                nc.vector.reduce_max(out=m2[:], in_=v3(exm[:]), axis=mybir.AxisListType.X)
                for b in range(NBLK):
                    nc.vector.tensor_scalar(
                        out=kp[:, b * E : (b + 1) * E],
                        in0=ex[:, b * E : (b + 1) * E],
                        scalar1=m2[:, b : b + 1], scalar2=None, op0=ALU.is_ge,
                    )
                nc.vector.tensor_mul(kp[:], kp[:], ex[:])
            den = smal.tile([128, NBLK], F32, tag="den")
            nc.vector.reduce_sum(out=den[:], in_=v3(kp[:]), axis=mybir.AxisListType.X)
            dinv = smal.tile([128, NBLK], F32, tag="dinv")
            nc.vector.reciprocal(dinv[:], den[:])
            st["kp"] = kp
            st["dinv"] = dinv
            return st

        def body():
            prev = None
            for s in range(n_stripes):
                prev = emit_stripe(s, prev)
            tail_a(prev)
            tail_relu(prev)
            tail_hp(prev)
            tail_b(prev)

        if time_loops > 1:
            with tc.For_i(0, time_loops, 1):
                body()
        else:
            body()
    nc.compile()
    return nc


def _prep_weights(Wr, br, Wd, bd, Wu, bu):
    """Host-side weight layout preprocessing (all tiny)."""
    import ml_dtypes

    bf16 = ml_dtypes.bfloat16
    Wr = np.asarray(Wr, np.float32)
    br = np.asarray(br, np.float32)
    Wd = np.asarray(Wd, np.float32)
    bd = np.asarray(bd, np.float32)
    Wu = np.asarray(Wu, np.float32)
    bu = np.asarray(bu, np.float32)
    # wds[p, c*128 + e*16 + r] = Wd[e, c*128+p, r]
    wds = np.ascontiguousarray(
        Wd.reshape(E, KC, 128, R).transpose(2, 1, 0, 3).reshape(128, KC * E * R)
    )
    # wrs[p, c*E + e] = Wr[c*128+p, e]
    wrs = np.ascontiguousarray(
        Wr.reshape(KC, 128, E).transpose(1, 0, 2).reshape(128, KC * E)
    )
    i128b = np.eye(128, dtype=bf16)
    i128r = np.eye(128, dtype=np.float32)
    wus = np.ascontiguousarray(Wu.reshape(ER, D)).astype(bf16)
    bus = np.ascontiguousarray(bu).astype(bf16)
    bds = np.ascontiguousarray(bd.reshape(ER, 1))
    brb = np.ascontiguousarray(np.tile(br, (128, NBLK)))
    sel_m = np.zeros((E, ER), bf16)
    for e in range(E):
        sel_m[e, e * R : (e + 1) * R] = 1.0
    csum = np.zeros((128, E), np.float32)
    for g in range(4):
        for j in range(E):
            csum[32 * g + j, j] = 1.0
    return dict(wds=wds, wrs=wrs, wus=wus, bus=bus, bds=bds, brb=brb, i128b=i128b, i128r=i128r, sel=sel_m, csum=csum)


_NC_CACHE = {}


def _get_program(t_core=T_CORE, fast_math=True):
    key = (t_core, fast_math)
    if key not in _NC_CACHE:
        _NC_CACHE[key] = _build_program_v2(t_core, fast_math)
    return _NC_CACHE[key]


_build_program = _build_program_v2


def kernel(x, Wr, br, Wd, bd, Wu, bu):
    from concourse.bass_utils import run_bass_kernel_spmd

    x = np.asarray(x, np.float32)
    wmap = _prep_weights(Wr, br, Wd, bd, Wu, bu)
    xf = np.ascontiguousarray(x.reshape(B * S, D))
    nc = _get_program()
    in_maps = []
    for i in range(N_CORES):
        m = dict(wmap)
        m["x"] = xf[i * T_CORE : (i + 1) * T_CORE]
        in_maps.append(m)
    res = run_bass_kernel_spmd(nc, in_maps, list(range(N_CORES)))
    outs = [res.results[i]["out"] for i in range(N_CORES)]
    return np.concatenate(outs, axis=0).reshape(B, S, D)


# revision 12
# speedup vs baseline: 1.1018x; 1.1018x over previous
"""MoE adapter (router + rank-16 expert adapters) Trainium2 Bass kernel, v2.

Math: with w[t,e] the dense (zero for non-top2) UNNORMALIZED top-2 gates
(kp = exp values of the top-2, zero elsewhere) and den[t] = sum_e kp[t,e]:
  out[t,:] = (1/den[t]) * [ (kp_expand ⊙ relu(x@WdFlat + bdFlat)) @ WuFlat + kp @ bu ]
The 1/den is folded into the PSUM->SBUF evacuation of the final output
(per-partition scalar multiply), so gates are never normalized explicitly.
exp() needs no max-subtraction: |logit| <= ||x_row||*||Wr_col|| + |br| < 40,
so exp stays finite in fp32 and top-2 ratios are shift-invariant anyway.

Sharding: pure data parallel, tokens split 8 ways, weights replicated.

Per-core, per 512-token stripe, software-pipelined across stripes
(front(s) = transpose/router/down; tail(s-1) = gate-consumers/up/bias/out):
  PE:  32 x-transposes (f32r), 8 router MMs, 8 down MMs, 4 logit transposes,
       4 kp transposes, 1 sel MM (gate broadcast), 16 up/bias MMs.
  ACT: 4 x^T evacs, relu+bias, exp, wt evac, plg evac, 4 out evacs.
  DVE: 4 x^T evacs, gate math (~8 small ops), hp gate-mul, 4 out evacs.
  DMA: 4x 512KB in, 4x 512KB out  (the ~11.2us/stripe roofline).
"""

import sys

sys.path.insert(0, "/opt/trn_rl_repo")

from contextlib import ExitStack

import numpy as np

import concourse.bacc as bacc
import concourse.bass as bass
import concourse.mybir as mybir
import concourse.tile as tile

F32 = mybir.dt.float32
F32R = mybir.dt.float32r
BF16 = mybir.dt.bfloat16
ALU = mybir.AluOpType
ACTF = mybir.ActivationFunctionType

B, S, D = 8, 4096, 1024
E, R, TOP_K = 8, 16, 2
ER = E * R  # 128
N_CORES = 8
T_CORE = B * S // N_CORES  # 4096 tokens per core
STRIPE = 512
NBLK = STRIPE // 128  # 4
KC = D // 128  # 8 k-chunks


def _build_program_v2(t_core: int = T_CORE, fast_math: bool = True, time_loops: int = 1, cfg: dict | None = None):
    nc = bacc.Bacc("TRN2", target_bir_lowering=False, debug=False)
    CF = {
        "xin_bufs": 8,
        "xtp_bufs": 2,
        "pbig_bufs": 3,
        "outp_bufs": 4,
        "smal_bufs": 2,
        "stt": True,          # use fused scalar_tensor_tensor for top-2 masks
        "act_scale_ap": True,  # ACT Copy with per-partition scale AP for out evac
        "skew": 2,             # chunks of evac lag before router/down start
        "evac_pat": "avavavav",  # xt-evac engine per chunk: a=ACT v=DVE
        "po_pat": "vavavava",    # out-evac engine per (blk,half)
        "bcast_mask": False,     # use broadcast-AP tensor_tensor for top-2 masks
        "dma_pair": False,       # pair 128-row blocks into 1MB DMAs
        "out_dma_eng": "sync",   # ring for out-DMAs: scalar=ACT-HWDGE, sync=SP
        "ct_router": False,      # col-tiled router: concurrent col-group MMs
        "ct_groups": 4,          # how many 32-col groups to use (3 or 4)
        "ta_c": 0,               # chunk index at which prev-stripe tail_a emits
        "hoist_relu": False,     # emit prev-stripe relu first in the ACT queue (measured worse)
    }
    CF.update(cfg or {})

    phase = CF.get("phase", "")
    x = nc.dram_tensor("x", [t_core, D], F32R, kind="ExternalInput").ap()
    wds = nc.dram_tensor("wds", [128, D], F32R, kind="ExternalInput").ap()
    wrs = nc.dram_tensor("wrs", [128, KC * E], F32R, kind="ExternalInput").ap()
    wus = nc.dram_tensor("wus", [ER, D], BF16, kind="ExternalInput").ap()
    bus = nc.dram_tensor("bus", [E, D], BF16, kind="ExternalInput").ap()
    bds = nc.dram_tensor("bds", [128, 1], F32, kind="ExternalInput").ap()
    brb = nc.dram_tensor("brb", [128, NBLK * E], F32, kind="ExternalInput").ap()
    i128b = nc.dram_tensor("i128b", [128, 128], BF16, kind="ExternalInput").ap()
    i128r = nc.dram_tensor("i128r", [128, 128], F32R, kind="ExternalInput").ap()
    sel = nc.dram_tensor("sel", [E, ER], BF16, kind="ExternalInput").ap()
    csum = nc.dram_tensor("csum", [128, E], F32R, kind="ExternalInput").ap()
    out = nc.dram_tensor(
        "out", [t_core, D], F32R if phase in ("T", "D") else F32, kind="ExternalOutput"
    ).ap()

    n_stripes = t_core // STRIPE
    assert t_core % STRIPE == 0

    with tile.TileContext(nc) as tc, ExitStack() as ctx:
        const = ctx.enter_context(tc.tile_pool(name="const", bufs=1))
        xin = ctx.enter_context(tc.tile_pool(name="xin", bufs=CF["xin_bufs"]))
        xtp = ctx.enter_context(tc.tile_pool(name="xt", bufs=CF["xtp_bufs"]))
        hsp = ctx.enter_context(tc.tile_pool(name="hs", bufs=2))
        hpp = ctx.enter_context(tc.tile_pool(name="hp", bufs=2))
        smal = ctx.enter_context(tc.tile_pool(name="smal", bufs=CF["smal_bufs"]))
        outp = ctx.enter_context(tc.tile_pool(name="outsb", bufs=CF["outp_bufs"]))
        # PSUM (8 banks): pbig 3 (x-transpose tiles & up-output tiles,
        # disjoint phases) + plg 1 + ph 2 + psmA 2 (pwt/pwb/plgtm).
        pbig = ctx.enter_context(tc.tile_pool(name="pbig", bufs=CF["pbig_bufs"], space="PSUM"))
        plgp = ctx.enter_context(tc.tile_pool(name="plg", bufs=1, space="PSUM"))
        php = ctx.enter_context(tc.tile_pool(name="ph", bufs=2, space="PSUM"))
        psmA = ctx.enter_context(tc.tile_pool(name="psmA", bufs=2, space="PSUM"))

        # ---- one-time constant loads ----
        i128b_t = const.tile([128, 128], BF16)
        nc.sync.dma_start(i128b_t[:], i128b)
        i128r_t = const.tile([128, 128], F32R)
        nc.sync.dma_start(i128r_t[:], i128r)
        def load_x(tok0):
            if CF["dma_pair"]:
                xts = []
                for p in range(NBLK // 2):
                    xb = xin.tile([128, 2 * D], F32R, tag="xin")
                    src = x[tok0 + p * 256 : tok0 + (p + 1) * 256, :].rearrange(
                        "(c t) d -> t c d", c=2
                    )
                    nc.sync.dma_start(xb[:].rearrange("t (c d) -> t c d", d=D), src)
                    xts.append(xb[:, 0:D])
                    xts.append(xb[:, D : 2 * D])
                return xts
            xts = []
            for b in range(NBLK):
                xb = xin.tile([128, D], F32R, tag="xin")
                nc.sync.dma_start(xb[:], x[tok0 + b * 128 : tok0 + (b + 1) * 128, :])
                xts.append(xb[:])
            return xts

        pre_x = []
        if time_loops == 1:
            pre_x.extend(load_x(0))
        wds_t = const.tile([128, D], F32R)
        nc.sync.dma_start(wds_t[:], wds)
        wrs_t = const.tile([128, KC * E], F32R)
        nc.sync.dma_start(wrs_t[:], wrs)
        wus_t = const.tile([ER, D], BF16)
        nc.sync.dma_start(wus_t[:], wus)
        bus_t = const.tile([E, D], BF16)
        nc.sync.dma_start(bus_t[:], bus)
        bds_t = const.tile([128, 1], F32)
        nc.sync.dma_start(bds_t[:], bds)
        brb_t = const.tile([128, NBLK * E], F32)
        nc.sync.dma_start(brb_t[:], brb)
        sel_t = const.tile([E, ER], BF16)
        nc.sync.dma_start(sel_t[:], sel)
        if CF["ct_router"]:
            csum_t = const.tile([128, E], F32R)
            nc.sync.dma_start(csum_t[:], csum)
            plg128 = plgp.tile([128, STRIPE], F32, tag="plg")
            nc.vector.memset(plg128[:], 0.0)

        def v3(ap):
            return ap.rearrange("p (b e) -> p b e", e=E)

        # ---------- tail of stripe sp (gate-consumers, up+bias, out) ----------
        # split into tail_a (emitted early in the next stripe's front) and
        # tail_b (up/bias matmuls + out, emitted after the next front's MMs).
        def tail_a(st):
            if st is None:
                return
            kp, dinv = st["kp"], st["dinv"]
            # kp^T blocks -> pwt [8, 512]
            pwt = psmA.tile([E, STRIPE], BF16, tag="sm")
            for b in range(NBLK):
                nc.tensor.transpose(
                    pwt[:, b * 128 : (b + 1) * 128],
                    kp[:, b * E : (b + 1) * E],
                    i128b_t[:],
                )
            wt = smal.tile([E, STRIPE], BF16, tag="wt")
            nc.scalar.copy(wt[:], pwt[:])
            # broadcast gates to er rows: pb[16e+r, t] = kp[t, e]
            pb = psmA.tile([128, STRIPE], F32, tag="sm")
            nc.tensor.matmul(pb[:], sel_t[:], wt[:], start=True, stop=True)
            st["wt"] = wt
            st["pb"] = pb

        def tail_relu(st):
            if st is None:
                return
            hs = hsp.tile([128, STRIPE], F32R)
            nc.scalar.activation(hs[:], st["ph"][:], ACTF.Relu, bias=bds_t[:, 0:1])
            st["hs"] = hs

        def tail_hp(st):
            if st is None:
                return
            hp = hpp.tile([128, STRIPE], BF16)
            nc.vector.tensor_mul(hp[:], st["hs"][:], st["pb"][:])
            st["hp"] = hp

        def tail_b(st):
            if st is None:
                return
            tok0 = st["s"] * STRIPE
            hp, wt, dinv = st["hp"], st["wt"], st["dinv"]
            pair = CF["dma_pair"]
            odma = nc.scalar.dma_start if CF["out_dma_eng"] == "scalar" else nc.sync.dma_start
            nb_per = 2 if pair else 1
            for bp in range(NBLK // nb_per):
                osb = outp.tile([128, nb_per * D], F32, tag="osb")
                for bi in range(nb_per):
                    b = bp * nb_per + bi
                    for h2 in range(2):
                        po = pbig.tile([128, 512], F32, tag="pt")
                        nc.tensor.matmul(
                            po[:],
                            hp[:, b * 128 : (b + 1) * 128],
                            wus_t[:, h2 * 512 : (h2 + 1) * 512],
                            start=True,
                            stop=False,
                            skip_group_check=True,
                        )
                        nc.tensor.matmul(
                            po[:],
                            wt[:, b * 128 : (b + 1) * 128],
                            bus_t[:, h2 * 512 : (h2 + 1) * 512],
                            start=False,
                            stop=True,
                            skip_group_check=True,
                        )
                        dst = osb[:, bi * D + h2 * 512 : bi * D + (h2 + 1) * 512]
                        dv = dinv[:, b : b + 1]
                        if CF["po_pat"][b * 2 + h2] == "v" or not CF["act_scale_ap"]:
                            nc.vector.tensor_scalar_mul(dst, po[:], dv)
                        else:
                            nc.scalar.activation(dst, po[:], ACTF.Copy, scale=dv)
                if pair:
                    dstap = out[tok0 + bp * 256 : tok0 + (bp + 1) * 256, :].rearrange(
                        "(c t) d -> t c d", c=2
                    )
                    odma(dstap, osb[:].rearrange("t (c d) -> t c d", d=D))
                else:
                    dstap = out[tok0 + bp * 128 : tok0 + (bp + 1) * 128, :]
                    odma(dstap, osb[:])

        # ---------- front of stripe s + interleaved tail of prev ----------
        def emit_stripe(s, prev):
            tok0 = s * STRIPE
            # x DMAs
            if s == 0 and pre_x:
                xts = pre_x[:]
                pre_x.clear()
            else:
                xts = load_x(tok0)

            if CF["hoist_relu"] and phase == "":
                tail_relu(prev)  # ACT: first in queue; ph(prev) already done

            if phase == "D":
                # DMA-only roofline microbenchmark: bounce x straight back out
                for b in range(NBLK):
                    nc.sync.dma_start(
                        out[tok0 + b * 128 : tok0 + (b + 1) * 128, :], xts[b]
                    )
                return None

            xt_all = xtp.tile([128, KC * STRIPE], F32R)

            def xtc(c):
                return xt_all[:, c * STRIPE : (c + 1) * STRIPE]

            ct = CF["ct_router"]
            G = CF["ct_groups"]
            if not ct:
                plg = plgp.tile([E, STRIPE], F32, tag="plg")
            ph = php.tile([128, STRIPE], F32, tag="ph")
            skew = CF["skew"]

            def router_round(r):
                # G concurrent col-group matmuls (chunks r*G .. r*G+G-1)
                for g in range(G):
                    c = r * G + g
                    if c >= KC:
                        return
                    nc.tensor.matmul(
                        plg128[32 * g : 32 * g + E, :],
                        wrs_t[:, c * E : (c + 1) * E],
                        xtc(c),
                        start=(c == 0),
                        stop=(c == KC - 1),
                        skip_group_check=True,
                        tile_position=(0, 32 * g),
                    )

            rounds_done = set()

            def mm_pair(c):
                if ct:
                    # router handled in rounds; emit a round once its chunks
                    # are all evacuated
                    if (c + 1) % G == 0:
                        router_round(c // G)
                        rounds_done.add(c // G)
                else:
                    nc.tensor.matmul(
                        plg[:],
                        wrs_t[:, c * E : (c + 1) * E],
                        xtc(c),
                        start=(c == 0),
                        stop=(c == KC - 1),
                        skip_group_check=True,
                    )
                nc.tensor.matmul(
                    ph[:],
                    wds_t[:, c * 128 : (c + 1) * 128],
                    xtc(c),
                    start=(c == 0),
                    stop=(c == KC - 1),
                    skip_group_check=True,
                )

            for c in range(KC):
                pt = pbig.tile([128, STRIPE], F32R, tag="pt")
                for b in range(NBLK):
                    nc.tensor.transpose(
                        pt[:, b * 128 : (b + 1) * 128],
                        xts[b][:, c * 128 : (c + 1) * 128],
                        i128r_t[:],
                    )
                # alternate evac engine
                if CF["evac_pat"][c] == "a":
                    nc.scalar.copy(xtc(c), pt[:])
                else:
                    nc.vector.tensor_copy(xtc(c), pt[:])
                if phase == "T":
                    continue
                if c == CF["ta_c"]:
                    tail_a(prev)       # PE: 4 kp-transposes + sel MM
                if c == CF["ta_c"] + 1 and not CF["hoist_relu"]:
                    tail_relu(prev)    # ACT
                if c == CF["ta_c"] + 2:
                    tail_hp(prev)      # DVE
                if c >= skew:
                    mm_pair(c - skew)
            if phase == "T":
                # microbenchmark: DMA out the transposed data directly
                # (same 4x512KB out traffic as the real pipeline; values garbage)
                for j in range(NBLK):
                    nc.sync.dma_start(
                        out[tok0 + j * 128 : tok0 + (j + 1) * 128, :],
                        xt_all[:, j * 2 * STRIPE : (j + 1) * 2 * STRIPE],
                    )
                return None
            for c in range(KC - skew, KC):
                mm_pair(c)

            if ct:
                for r in range((KC + G - 1) // G):
                    if r not in rounds_done:
                        router_round(r)

            st = {"s": s, "ph": ph}

            # evacuate router logits NOW so the ACT op sits ahead of the
            # prev-stripe out-evacs in the queue; the PE-side logit
            # transposes stay after tail_b (no PE stall, gate chain ~1.2us
            # earlier)
            if CF.get("hoist_lgt", True):
                if ct:
                    lgt = smal.tile([128, STRIPE], F32R, tag="lgt")
                    nc.scalar.copy(lgt[:], plg128[:])
                else:
                    lgt = smal.tile([E, STRIPE], F32R, tag="lgt")
                    nc.scalar.copy(lgt[:], plg[:])
                lgt_done = True
            else:
                lgt_done = False

            # up/bias + out of previous stripe (PE keeps busy while this
            # stripe's logits are evacuated and gated)
            tail_b(prev)

            # ---- router logits -> token-major ----
            if ct:
                if not lgt_done:
                    lgt = smal.tile([128, STRIPE], F32R, tag="lgt")
                    nc.scalar.copy(lgt[:], plg128[:])
                plgtm = psmA.tile([128, NBLK * E], F32, tag="sm")
                for b in range(NBLK):
                    nc.tensor.matmul(
                        plgtm[:, b * E : (b + 1) * E],
                        lgt[:, b * 128 : (b + 1) * 128],
                        csum_t[:],
                        start=True,
                        stop=True,
                        skip_group_check=True,
                    )
            else:
                if not lgt_done:
                    lgt = smal.tile([E, STRIPE], F32R, tag="lgt")
                    nc.scalar.copy(lgt[:], plg[:])
                plgtm = psmA.tile([128, NBLK * E], F32R, tag="sm")
                for b in range(NBLK):
                    nc.tensor.transpose(
                        plgtm[:, b * E : (b + 1) * E],
                        lgt[0:E, b * 128 : (b + 1) * 128],
                        i128r_t[:E, :E],
                    )

            # ---- top-2 gate math (unnormalized; 1/den deferred to out evac) ----
            lg2 = smal.tile([128, NBLK * E], F32, tag="lg2")
            nc.vector.tensor_add(lg2[:], plgtm[:], brb_t[:])
            ex = smal.tile([128, NBLK * E], F32, tag="ex")
            nc.scalar.activation(ex[:], lg2[:], ACTF.Exp)
            m1 = smal.tile([128, NBLK], F32, tag="m1")
            nc.vector.reduce_max(out=m1[:], in_=v3(ex[:]), axis=mybir.AxisListType.X)
            exm = smal.tile([128, NBLK * E], F32, tag="exm")
            kp = smal.tile([128, NBLK * E], BF16, tag="kp")
            m2 = smal.tile([128, NBLK], F32, tag="m2")
            if CF["bcast_mask"]:
                m1b = m1[:].unsqueeze(2).broadcast_to([128, NBLK, E])
                nc.vector.tensor_tensor(
                    out=v3(exm[:]), in0=v3(ex[:]), in1=m1b, op=ALU.is_lt
                )
                nc.vector.tensor_mul(exm[:], exm[:], ex[:])
                nc.vector.reduce_max(out=m2[:], in_=v3(exm[:]), axis=mybir.AxisListType.X)
                m2b = m2[:].unsqueeze(2).broadcast_to([128, NBLK, E])
                nc.vector.tensor_tensor(
                    out=v3(kp[:]), in0=v3(ex[:]), in1=m2b, op=ALU.is_ge
                )
                nc.vector.tensor_mul(kp[:], kp[:], ex[:])
            elif CF["stt"]:
                for b in range(NBLK):
                    eb = ex[:, b * E : (b + 1) * E]
                    nc.vector.scalar_tensor_tensor(
                        exm[:, b * E : (b + 1) * E], eb, m1[:, b : b + 1], eb,
                        op0=ALU.is_lt, op1=ALU.mult,
                    )
                nc.vector.reduce_max(out=m2[:], in_=v3(exm[:]), axis=mybir.AxisListType.X)
                for b in range(NBLK):
                    eb = ex[:, b * E : (b + 1) * E]
                    nc.vector.scalar_tensor_tensor(
                        kp[:, b * E : (b + 1) * E], eb, m2[:, b : b + 1], eb,
                        op0=ALU.is_ge, op1=ALU.mult,
                    )
            else:
                for b in range(NBLK):
                    nc.vector.tensor_scalar(
                        out=exm[:, b * E : (b + 1) * E],
                        in0=ex[:, b * E : (b + 1) * E],
                        scalar1=m1[:, b : b + 1], scalar2=None, op0=ALU.is_lt,
                    )
                nc.vector.tensor_mul(exm[:], exm[:], ex[:])
                nc.vector.reduce_max(out=m2[:], in_=v3(exm[:]), axis=mybir.AxisListType.X)
                for b in range(NBLK):
                    nc.vector.tensor_scalar(
                        out=kp[:, b * E : (b + 1) * E],
                        in0=ex[:, b * E : (b + 1) * E],
                        scalar1=m2[:, b : b + 1], scalar2=None, op0=ALU.is_ge,
                    )
                nc.vector.tensor_mul(kp[:], kp[:], ex[:])
            den = smal.tile([128, NBLK], F32, tag="den")
            nc.vector.reduce_sum(out=den[:], in_=v3(kp[:]), axis=mybir.AxisListType.X)
            dinv = smal.tile([128, NBLK], F32, tag="dinv")
            nc.vector.reciprocal(dinv[:], den[:])
            st["kp"] = kp
            st["dinv"] = dinv
            return st

        def body():
            prev = None
            for s in range(n_stripes):
                prev = emit_stripe(s, prev)
            tail_a(prev)
            tail_relu(prev)
            tail_hp(prev)
            tail_b(prev)

        if time_loops > 1:
            with tc.For_i(0, time_loops, 1):
                body()
        else:
            body()
    nc.compile()
    return nc


def _prep_weights(Wr, br, Wd, bd, Wu, bu):
    """Host-side weight layout preprocessing (all tiny)."""
    import ml_dtypes

    bf16 = ml_dtypes.bfloat16
    Wr = np.asarray(Wr, np.float32)
    br = np.asarray(br, np.float32)
    Wd = np.asarray(Wd, np.float32)
    bd = np.asarray(bd, np.float32)
    Wu = np.asarray(Wu, np.float32)
    bu = np.asarray(bu, np.float32)
    # wds[p, c*128 + e*16 + r] = Wd[e, c*128+p, r]
    wds = np.ascontiguousarray(
        Wd.reshape(E, KC, 128, R).transpose(2, 1, 0, 3).reshape(128, KC * E * R)
    )
    # wrs[p, c*E + e] = Wr[c*128+p, e]
    wrs = np.ascontiguousarray(
        Wr.reshape(KC, 128, E).transpose(1, 0, 2).reshape(128, KC * E)
    )
    i128b = np.eye(128, dtype=bf16)
    i128r = np.eye(128, dtype=np.float32)
    wus = np.ascontiguousarray(Wu.reshape(ER, D)).astype(bf16)
    bus = np.ascontiguousarray(bu).astype(bf16)
    bds = np.ascontiguousarray(bd.reshape(ER, 1))
    brb = np.ascontiguousarray(np.tile(br, (128, NBLK)))
    sel_m = np.zeros((E, ER), bf16)
    for e in range(E):
        sel_m[e, e * R : (e + 1) * R] = 1.0
    csum = np.zeros((128, E), np.float32)
    for g in range(4):
        for j in range(E):
            csum[32 * g + j, j] = 1.0
    return dict(wds=wds, wrs=wrs, wus=wus, bus=bus, bds=bds, brb=brb, i128b=i128b, i128r=i128r, sel=sel_m, csum=csum)


_NC_CACHE = {}


def _get_program(t_core=T_CORE, fast_math=True):
    key = (t_core, fast_math)
    if key not in _NC_CACHE:
        _NC_CACHE[key] = _build_program_v2(t_core, fast_math)
    return _NC_CACHE[key]


_build_program = _build_program_v2


def kernel(x, Wr, br, Wd, bd, Wu, bu):
    from concourse.bass_utils import run_bass_kernel_spmd

    x = np.asarray(x, np.float32)
    wmap = _prep_weights(Wr, br, Wd, bd, Wu, bu)
    xf = np.ascontiguousarray(x.reshape(B * S, D))
    nc = _get_program()
    in_maps = []
    for i in range(N_CORES):
        m = dict(wmap)
        m["x"] = xf[i * T_CORE : (i + 1) * T_CORE]
        in_maps.append(m)
    res = run_bass_kernel_spmd(nc, in_maps, list(range(N_CORES)))
    outs = [res.results[i]["out"] for i in range(N_CORES)]
    return np.concatenate(outs, axis=0).reshape(B, S, D)
